# revision 3
# baseline (speedup 1.0000x reference)
"""Multi-head attention forward on 8 Trainium2 NeuronCores.

Problem: batch=8, seq=1024, d_model=1024, n_heads=16, d_head=64, fp32 ref.

Sharding: data-parallel over batch - core b computes batch element b end to
end (weights replicated, no collectives).

Per-core layout strategy (nothing ever needs an on-device transpose):
  - x^T (d on partitions) is staged by the host; it serves as
      rhs  for Q^T/K^T = W^T @ x^T   (2 heads packed -> M=128)
      lhsT for V      = x @ W_V      (heads along the free dim)
  - scores^T = K @ Q^T lands with k on partitions, so softmax's exp is one
    ScalarE activation per tile (the 1/sqrt(d) scale and the key-mask fold
    in as activation scale/bias), and the sum over k happens inside the
    P@V matmul via a ones-column appended to V (softmax denominators pop
    out in psum row 64 for free).
  - Z^T = [V|1]^T @ P^T keeps (head, e) on partitions; heads are packed in
    pairs so the output projection contracts with K=128.
  - biases are folded into the DVE psum->sbuf evacuation ops (per-partition
    tensor_scalar for Q/K, broadcast-staged tensor_tensor for V and the
    output projection) so they cost zero TensorE cycles.

Schedule: one software-pipelined loop over head pairs keeps all engines
busy simultaneously instead of running projection / attention / output
phases back to back:

  pre    : load DMAs, Q^T/K^T projection for pair 0
  iter 0 : scores+exp pair 0 interleaved with the whole V projection,
           then Q^T/K^T pair 1
  iter g : scores+exp pair g interleaved with PV(pair g-1) and the
           Q^T/K^T projection of pair g+1
  tail   : PV(pair 7), output projection

The two heads of a pair have d_head=64 so their score matmuls occupy
disjoint PE row groups (partitions 0-63 / 64-127); emitting them
back-to-back lets the PE run them concurrently (row tiling), and one
N=1024 ScalarE activation then exps both heads' scores at once.
Keeping the PE stream dense also keeps the HAM clock gate at 2.4 GHz
(the phase-serialized version stalled >3.4us on every head and ran the
whole attention phase at the cold 1.2 GHz clock).

PSUM budget (8 banks): 2 proj + 4 scores (2 tiles x 2 banks) + 2 PV.

Everything is bf16 into the PE with fp32 PSUM accumulation.

This toolchain's walrus encodes at most ONE sync wait per instruction, so
_split_multi_waits hoists excess waits onto same-engine EventSemaphore
instructions (engines execute their streams in order, so this is exact).
"""

from contextlib import ExitStack

import numpy as np

import concourse.bass as bass
import concourse.tile as tile
from concourse import mybir
from concourse.bass_utils import run_bass_kernel_spmd

S = 1024  # seq
D = 1024  # d_model
H = 16  # heads
E = 64  # d_head
B = 8  # batch == n_cores
P = 128  # partitions
NS = S // P  # 8 s-tiles
ND = D // P  # 8 d-chunks
NG = H // 2  # 8 head pairs

F32 = mybir.dt.float32
BF16 = mybir.dt.bfloat16
AF = mybir.ActivationFunctionType

MASK_NEG = 60.0  # exp(x - 60) ~ 9e-27: masked keys vanish without inf/nan


def build_program(split_waits=True):
    nc = bass.Bass("TRN2", target_bir_lowering=False, debug=False)

    # all inputs arrive pre-packed by the host into their exact SBUF layouts
    xt_d = nc.dram_tensor("xt", [P, ND, S], BF16, kind="ExternalInput").ap()
    wq_d = nc.dram_tensor("wq", [P, NG, ND, P], BF16, kind="ExternalInput").ap()
    wk_d = nc.dram_tensor("wk", [P, NG, ND, P], BF16, kind="ExternalInput").ap()
    wv_d = nc.dram_tensor("wv", [P, ND, H * E], BF16, kind="ExternalInput").ap()
    wo_d = nc.dram_tensor("wo", [P, NG, D], BF16, kind="ExternalInput").ap()
    # per-partition Q/K biases: col g = b_Q[pair g], col NG+g = b_K[pair g]
    bqk_d = nc.dram_tensor("bqk", [P, 2 * NG], F32, kind="ExternalInput").ap()
    # partition-broadcast b_V (h,e) and b_O (d)
    bvb_d = nc.dram_tensor("bvb", [P, H * E], BF16, kind="ExternalInput").ap()
    bob_d = nc.dram_tensor("bob", [P, D], BF16, kind="ExternalInput").ap()
    mb_d = nc.dram_tensor("mb", [P, NS], F32, kind="ExternalInput").ap()
    out_d = nc.dram_tensor("out", [S, D], F32, kind="ExternalOutput").ap()

    with tile.TileContext(nc) as tc, ExitStack() as ctx:
        g1 = ctx.enter_context(tc.tile_pool(name="g1", bufs=1))
        wqkp = ctx.enter_context(tc.tile_pool(name="wqk", bufs=4))
        ptp = ctx.enter_context(tc.tile_pool(name="ptp", bufs=28))
        rcp = ctx.enter_context(tc.tile_pool(name="rcp", bufs=4))
        bcp = ctx.enter_context(tc.tile_pool(name="bcp", bufs=4))
        obp = ctx.enter_context(tc.tile_pool(name="obp", bufs=2))
        # PSUM: exactly 8 banks
        pp = ctx.enter_context(tc.tile_pool(name="pp", bufs=2, space="PSUM"))
        stp = ctx.enter_context(tc.tile_pool(name="stp", bufs=2, space="PSUM"))
        zpsp = ctx.enter_context(tc.tile_pool(name="zps", bufs=2, space="PSUM"))

        # ---- input DMAs (most-urgent first; independent queues) ----
        wqk_t = {}

        def load_pair(g):
            wq_t = wqkp.tile([P, ND, P], BF16, tag="wq_t", name=f"wq{g}")
            wk_t = wqkp.tile([P, ND, P], BF16, tag="wk_t", name=f"wk{g}")
            nc.sync.dma_start(out=wq_t, in_=wq_d[:, g])
            nc.sync.dma_start(out=wk_t, in_=wk_d[:, g])
            wqk_t[g] = (wq_t, wk_t)

        xT = g1.tile([P, ND, S], BF16, tag="xT")
        for c in range(ND):
            nc.sync.dma_start(out=xT[:, c], in_=xt_d[:, c])
        load_pair(0)
        mb_sb = g1.tile([P, NS], F32, tag="mb")
        nc.sync.dma_start(out=mb_sb, in_=mb_d)
        bqk_sb = g1.tile([P, 2 * NG], F32, tag="bqk")
        nc.sync.dma_start(out=bqk_sb, in_=bqk_d)
        load_pair(1)
        wv_sb = g1.tile([P, ND, H * E], BF16, tag="wv_sb")
        for c in range(ND):
            nc.sync.dma_start(out=wv_sb[:, c], in_=wv_d[:, c])
        bvb_sb = g1.tile([P, H * E], BF16, tag="bvb")
        nc.sync.dma_start(out=bvb_sb, in_=bvb_d)
        bob_sb = g1.tile([P, D], BF16, tag="bob")
        nc.sync.dma_start(out=bob_sb, in_=bob_d)
        wo_sb = g1.tile([P, NG, D], BF16, tag="wo_sb")
        nc.sync.dma_start(out=wo_sb, in_=wo_d)

        # persistent activations
        qT = g1.tile([P, NG, S], BF16, tag="qT")
        kT = g1.tile([P, NG, S], BF16, tag="kT")
        vb = g1.tile([P, NS, H, E + 1], BF16, tag="vb")
        zT = g1.tile([P, NG, S], BF16, tag="zT")
        # softmax-sum ones columns (V proj fills cols 0..E-1)
        nc.vector.memset(vb[:, :, :, E : E + 1], 1.0)

        pt_tiles = {}

        def qk_half(g, which):
            ti = 0 if which == "q" else 1
            dst = qT if which == "q" else kT
            w_t = wqk_t[g][ti]
            bcol = g if which == "q" else NG + g
            ps = [
                pp.tile([P, 512], F32, tag="pp", name=f"qk{g}{ti}{i}")
                for i in range(2)
            ]
            for c in range(ND):
                for qh in range(2):  # same lhsT back-to-back
                    nc.tensor.matmul(
                        out=ps[qh],
                        lhsT=w_t[:, c],
                        rhs=xT[:, c, qh * 512 : (qh + 1) * 512],
                        start=(c == 0),
                        stop=(c == ND - 1),
                    )
            for qh in range(2):
                nc.vector.tensor_scalar_add(
                    out=dst[:, g, qh * 512 : (qh + 1) * 512],
                    in0=ps[qh],
                    scalar1=bqk_sb[:, bcol : bcol + 1],
                )

        def v_proj(st):
            ps = [
                pp.tile([P, 512], F32, tag="pp", name=f"v{st}{i}")
                for i in range(2)
            ]
            for c in range(ND):
                for hh in range(2):  # same lhsT back-to-back
                    nc.tensor.matmul(
                        out=ps[hh],
                        lhsT=xT[:, c, st * P : (st + 1) * P],
                        rhs=wv_sb[:, c, hh * 512 : (hh + 1) * 512],
                        start=(c == 0),
                        stop=(c == ND - 1),
                    )
            for hh in range(2):
                nc.vector.tensor_add(
                    out=vb[:, st, hh * 8 : (hh + 1) * 8, 0:E],
                    in0=ps[hh].rearrange("p (h e) -> p h e", h=8),
                    in1=bvb_sb[:, hh * 512 : (hh + 1) * 512].rearrange(
                        "p (h e) -> p h e", h=8
                    ),
                )

        def sc(g, kt, qh):
            # the two heads' matmuls sit in disjoint PE row groups -> they
            # run concurrently; one N=1024 activation exps both
            stt = stp.tile([P, 2, 512], F32, tag="st", name=f"st{g}{kt}{qh}")
            ptt = ptp.tile([P, 2, 512], BF16, tag="pt", name=f"pt{g}{kt}{qh}")
            for h2 in range(2):
                nc.tensor.matmul(
                    out=stt[:, h2],
                    lhsT=kT[h2 * E : (h2 + 1) * E, g, kt * P : (kt + 1) * P],
                    rhs=qT[h2 * E : (h2 + 1) * E, g, qh * 512 : (qh + 1) * 512],
                    start=True,
                    stop=True,
                )
            nc.scalar.activation(
                out=ptt,
                in_=stt,
                func=AF.Exp,
                bias=mb_sb[:, kt : kt + 1],
                scale=0.125,
            )
            pt_tiles[(g, kt, qh)] = ptt

        def pv(g, h2):
            h = 2 * g + h2
            zp2 = [
                zpsp.tile([E + 1, 512], F32, tag="zp", name=f"zp{g}{h2}{i}")
                for i in range(2)
            ]
            for kt in range(NS):
                for qh in range(2):  # same lhsT back-to-back
                    nc.tensor.matmul(
                        out=zp2[qh],
                        lhsT=vb[:, kt, h, :],
                        rhs=pt_tiles[(g, kt, qh)][:, h2],
                        start=(kt == 0),
                        stop=(kt == NS - 1),
                    )
            for qh in range(2):
                zp = zp2[qh]
                rc = rcp.tile([1, 512], BF16, tag="rc", name=f"rc{g}{h2}{qh}")
                with nc.allow_low_precision(reason="bf16 softmax denom"):
                    nc.vector.reciprocal(out=rc, in_=zp[E : E + 1, :])
                bc = bcp.tile([E, 512], BF16, tag="bc", name=f"bc{g}{h2}{qh}")
                nc.sync.dma_start(
                    out=bc, in_=rc.unsqueeze(1).broadcast_to((1, E, 512))
                )
                nc.vector.tensor_mul(
                    zT[h2 * E : (h2 + 1) * E, g, qh * 512 : (qh + 1) * 512],
                    zp[0:E, :],
                    bc,
                )

        def out_proj(st):
            ops = [
                pp.tile([P, 512], F32, tag="pp", name=f"op{st}{i}")
                for i in range(2)
            ]
            for g in range(NG):
                for dh in range(2):  # same lhsT back-to-back
                    nc.tensor.matmul(
                        out=ops[dh],
                        lhsT=zT[:, g, st * P : (st + 1) * P],
                        rhs=wo_sb[:, g, dh * 512 : (dh + 1) * 512],
                        start=(g == 0),
                        stop=(g == NG - 1),
                    )
            ob = obp.tile([P, D], F32, tag="ob", name=f"ob{st}")
            for dh in range(2):
                nc.vector.tensor_add(
                    out=ob[:, dh * 512 : (dh + 1) * 512],
                    in0=ops[dh],
                    in1=bob_sb[:, dh * 512 : (dh + 1) * 512],
                )
            nc.sync.dma_start(out=out_d[st * P : (st + 1) * P, :], in_=ob)

        # ---- pipelined schedule ----
        qk_half(0, "q")
        qk_half(0, "k")
        load_pair(2)
        for kt in range(NS):  # iter 0: scores pair 0 + whole V projection
            sc(0, kt, 0)
            sc(0, kt, 1)
            v_proj(kt)
        qk_half(1, "q")
        qk_half(1, "k")
        for g in range(1, NG):
            if g + 2 <= NG - 1:
                load_pair(g + 2)
            sc(g, 0, 0); sc(g, 0, 1); sc(g, 1, 0); sc(g, 1, 1)
            pv(g - 1, 0)
            sc(g, 2, 0); sc(g, 2, 1); sc(g, 3, 0); sc(g, 3, 1)
            if g < NG - 1:
                qk_half(g + 1, "q")
            sc(g, 4, 0); sc(g, 4, 1); sc(g, 5, 0); sc(g, 5, 1)
            pv(g - 1, 1)
            sc(g, 6, 0); sc(g, 6, 1); sc(g, 7, 0); sc(g, 7, 1)
            if g < NG - 1:
                qk_half(g + 1, "k")
        pv(NG - 1, 0)
        pv(NG - 1, 1)
        for st in range(NS):
            out_proj(st)

    if split_waits:
        _split_multi_waits(nc)
    return nc


def _split_multi_waits(nc):
    """This walrus build encodes at most ONE sync wait per instruction.
    Tile emits more. Hoist excess waits onto same-engine EventSemaphore
    instructions inserted immediately before the offender - engines and
    DGE sequencers execute their streams in order, so this preserves
    semantics exactly."""
    n = 0
    for fn in nc.m.functions:
        for bb in fn.blocks:
            out = []
            for inst in bb.instructions:
                si = getattr(inst, "sync_info", None)
                waits = list(si.on_wait) if si is not None and si.on_wait else []
                if len(waits) > 1:
                    for w in waits[:-1]:
                        n += 1
                        out.append(
                            mybir.InstEventSemaphore(
                                name=f"evw-{n}",
                                engine=inst.engine,
                                sync_info=mybir.SyncInfo(
                                    on_wait=[w], on_update=[]
                                ),
                            )
                        )
                    si.on_wait = [waits[-1]]
                out.append(inst)
            bb.instructions[:] = out


_NC_CACHE = None


def _get_nc():
    global _NC_CACHE
    if _NC_CACHE is None:
        _NC_CACHE = build_program()
    return _NC_CACHE


def _make_in_maps(inputs):
    import ml_dtypes

    bf16 = ml_dtypes.bfloat16
    x = np.asarray(inputs["x"], np.float32)
    mask = np.asarray(inputs["key_attention_mask"])
    wq = np.asarray(inputs["W_Q"], np.float32).astype(bf16)
    wk = np.asarray(inputs["W_K"], np.float32).astype(bf16)
    wv = np.asarray(inputs["W_V"], np.float32).astype(bf16)
    wo = np.asarray(inputs["W_O"], np.float32).astype(bf16)
    bq = np.asarray(inputs["b_Q"], np.float32)  # (H, E)
    bk = np.asarray(inputs["b_K"], np.float32)
    bv = np.asarray(inputs["b_V"], np.float32)
    bo = np.asarray(inputs["b_O"], np.float32)  # (D,)

    def pack_qk(w):  # (H, D, E) -> [p, g, c, (h2 e)]
        return np.ascontiguousarray(
            w.reshape(NG, 2, ND, P, E).transpose(3, 0, 2, 1, 4).reshape(P, NG, ND, P)
        )

    # per-partition (half*64+e) bias columns per pair
    def pack_b(b):  # (H, E) -> [p, g]
        return np.ascontiguousarray(
            b.reshape(NG, 2, E).transpose(1, 2, 0).reshape(P, NG)
        )

    bqk = np.concatenate([pack_b(bq), pack_b(bk)], axis=1).astype(np.float32)
    shared = {
        "wq": pack_qk(wq),
        "wk": pack_qk(wk),
        # (H, D, E) -> [p, c, (h e)]
        "wv": np.ascontiguousarray(
            wv.reshape(H, ND, P, E).transpose(2, 1, 0, 3).reshape(P, ND, H * E)
        ),
        # (H, E, D) -> [(h2 e), g, d]
        "wo": np.ascontiguousarray(
            wo.reshape(NG, 2, E, D).transpose(1, 2, 0, 3).reshape(P, NG, D)
        ),
        "bqk": bqk,
        "bvb": np.ascontiguousarray(
            np.tile(bv.reshape(1, H * E), (P, 1))
        ).astype(bf16),
        "bob": np.ascontiguousarray(np.tile(bo.reshape(1, D), (P, 1))).astype(
            bf16
        ),
    }
    in_maps = []
    for b in range(B):
        m = dict(shared)
        xt = x[b].T.astype(bf16)  # (D, S) -> [p, c, s]
        m["xt"] = np.ascontiguousarray(
            xt.reshape(ND, P, S).transpose(1, 0, 2)
        )
        mb = ((mask[b] != 0).astype(np.float32) - 1.0) * MASK_NEG
        m["mb"] = np.ascontiguousarray(mb.reshape(NS, P).T)
        in_maps.append(m)
    return in_maps


def run(inputs, trace=False):
    nc = _get_nc()
    res = run_bass_kernel_spmd(nc, _make_in_maps(inputs), list(range(B)),
                               trace=trace)
    out = np.stack([res.results[b]["out"] for b in range(B)], axis=0)
    return out, res


def kernel(**inputs) -> np.ndarray:
    out, _ = run(inputs, trace=False)
    return out


# revision 13
# speedup vs baseline: 1.0391x; 1.0391x over previous
"""Multi-head attention forward on 8 Trainium2 NeuronCores.

Problem: batch=8, seq=1024, d_model=1024, n_heads=16, d_head=64, fp32 ref.

Sharding: data-parallel over batch - core b computes batch element b end to
end (weights replicated, no collectives).

Per-core layout strategy (nothing ever needs an on-device transpose):
  - x^T (d on partitions) is staged by the host; it serves as
      rhs  for Q^T/K^T = W^T @ x^T   (2 heads packed -> M=128)
      lhsT for V      = x @ W_V      (heads along the free dim)
  - scores^T = K @ Q^T lands with k on partitions, so softmax's exp is one
    ScalarE activation per tile (the 1/sqrt(d) scale and the key-mask fold
    in as activation scale/bias), and the sum over k happens inside the
    P@V matmul via a ones-column appended to V (softmax denominators pop
    out in psum row 64 for free).
  - Z^T = [V|1]^T @ P^T keeps (head, e) on partitions; heads are packed in
    pairs so the output projection contracts with K=128.
  - biases are folded into the DVE psum->sbuf evacuation ops (per-partition
    tensor_scalar for Q/K, broadcast-staged tensor_tensor for V and the
    output projection) so they cost zero TensorE cycles.

Schedule: one software-pipelined loop over head pairs keeps all engines
busy simultaneously instead of running projection / attention / output
phases back to back:

  pre    : load DMAs, Q^T/K^T projection for pair 0
  iter 0 : scores+exp pair 0 interleaved with the whole V projection,
           then Q^T/K^T pair 1
  iter g : scores+exp pair g interleaved with PV(pair g-1) and the
           Q^T/K^T projection of pair g+1
  tail   : PV(pair 7), output projection

The two heads of a pair have d_head=64 so their score matmuls occupy
disjoint PE row groups (partitions 0-63 / 64-127); emitting them
back-to-back lets the PE run them concurrently (row tiling), and one
N=1024 ScalarE activation then exps both heads' scores at once.
Keeping the PE stream dense also keeps the HAM clock gate at 2.4 GHz
(the phase-serialized version stalled >3.4us on every head and ran the
whole attention phase at the cold 1.2 GHz clock).

PSUM budget (8 banks): 2 proj + 4 scores (2 tiles x 2 banks) + 2 PV.

Everything is bf16 into the PE with fp32 PSUM accumulation.

This toolchain's walrus encodes at most ONE sync wait per instruction, so
_split_multi_waits hoists excess waits onto same-engine EventSemaphore
instructions (engines execute their streams in order, so this is exact).
"""

from contextlib import ExitStack

import numpy as np

import concourse.bass as bass
import concourse.tile as tile
from concourse import mybir
from concourse.bass_utils import run_bass_kernel_spmd

S = 1024  # seq
D = 1024  # d_model
H = 16  # heads
E = 64  # d_head
B = 8  # batch == n_cores
P = 128  # partitions
NS = S // P  # 8 s-tiles
ND = D // P  # 8 d-chunks
NG = H // 2  # 8 head pairs

F32 = mybir.dt.float32
BF16 = mybir.dt.bfloat16
AF = mybir.ActivationFunctionType

MASK_NEG = 60.0  # exp(x - 60) ~ 9e-27: masked keys vanish without inf/nan


def build_program(split_waits=True):
    nc = bass.Bass("TRN2", target_bir_lowering=False, debug=False)

    # all inputs arrive pre-packed by the host into their exact SBUF layouts
    xt_d = nc.dram_tensor("xt", [P, ND, S], BF16, kind="ExternalInput").ap()
    wq_d = nc.dram_tensor("wq", [P, NG, ND, P], BF16, kind="ExternalInput").ap()
    wk_d = nc.dram_tensor("wk", [P, NG, ND, P], BF16, kind="ExternalInput").ap()
    wv_d = nc.dram_tensor("wv", [P, ND, H * E], BF16, kind="ExternalInput").ap()
    wo_d = nc.dram_tensor("wo", [P, NG, D], BF16, kind="ExternalInput").ap()
    # per-partition Q/K biases: col g = b_Q[pair g], col NG+g = b_K[pair g]
    bqk_d = nc.dram_tensor("bqk", [P, 2 * NG], F32, kind="ExternalInput").ap()
    # partition-broadcast b_V (h,e) and b_O (d)
    bvb_d = nc.dram_tensor("bvb", [P, H * E], BF16, kind="ExternalInput").ap()
    bob_d = nc.dram_tensor("bob", [P, D], BF16, kind="ExternalInput").ap()
    mb_d = nc.dram_tensor("mb", [P, NS], F32, kind="ExternalInput").ap()
    out_d = nc.dram_tensor("out", [S, D], F32, kind="ExternalOutput").ap()

    with tile.TileContext(nc) as tc, ExitStack() as ctx:
        g1 = ctx.enter_context(tc.tile_pool(name="g1", bufs=1))
        wqkp = ctx.enter_context(tc.tile_pool(name="wqk", bufs=4))
        ptp = ctx.enter_context(tc.tile_pool(name="ptp", bufs=12))
        zup = ctx.enter_context(tc.tile_pool(name="zup", bufs=4))
        rcp = ctx.enter_context(tc.tile_pool(name="rcp", bufs=2))
        bcp = ctx.enter_context(tc.tile_pool(name="bcp", bufs=2))
        obp = ctx.enter_context(tc.tile_pool(name="obp", bufs=2))
        # PSUM: exactly 8 banks (2 proj + 4 scores + 2 PV)
        pp = ctx.enter_context(tc.tile_pool(name="pp", bufs=2, space="PSUM"))
        stp = ctx.enter_context(tc.tile_pool(name="stp", bufs=1, space="PSUM"))
        zpsp = ctx.enter_context(tc.tile_pool(name="zps", bufs=2, space="PSUM"))

        # ---- input DMAs (most-urgent first; independent queues) ----
        wqk_t = {}

        def load_pair(g):
            wq_t = wqkp.tile([P, ND, P], BF16, tag="wq_t", name=f"wq{g}")
            wk_t = wqkp.tile([P, ND, P], BF16, tag="wk_t", name=f"wk{g}")
            nc.sync.dma_start(out=wq_t, in_=wq_d[:, g])
            nc.sync.dma_start(out=wk_t, in_=wk_d[:, g])
            wqk_t[g] = (wq_t, wk_t)

        xT = g1.tile([P, ND, S], BF16, tag="xT")
        for c in range(ND):
            nc.sync.dma_start(out=xT[:, c], in_=xt_d[:, c])
        load_pair(0)
        mb_sb = g1.tile([P, NS], F32, tag="mb")
        nc.sync.dma_start(out=mb_sb, in_=mb_d)
        bqk_sb = g1.tile([P, 2 * NG], F32, tag="bqk")
        nc.sync.dma_start(out=bqk_sb, in_=bqk_d)
        load_pair(1)
        wv_sb = g1.tile([P, ND, H * E], BF16, tag="wv_sb")
        for c in range(ND):
            nc.sync.dma_start(out=wv_sb[:, c], in_=wv_d[:, c])
        bvb_sb = g1.tile([P, H * E], BF16, tag="bvb")
        nc.sync.dma_start(out=bvb_sb, in_=bvb_d)
        bob_sb = g1.tile([P, D], BF16, tag="bob")
        nc.sync.dma_start(out=bob_sb, in_=bob_d)
        wo_sb = g1.tile([P, NG, D], BF16, tag="wo_sb")
        nc.sync.dma_start(out=wo_sb, in_=wo_d)

        # persistent activations
        qT = g1.tile([P, NG, S], BF16, tag="qT")
        kT = g1.tile([P, NG, S], BF16, tag="kT")
        vb = g1.tile([P, NS, H, E + 1], BF16, tag="vb")
        zT = g1.tile([P, NG, S], BF16, tag="zT")
        # softmax-sum ones columns (V proj fills cols 0..E-1)
        nc.vector.memset(vb[:, :, :, E : E + 1], 1.0)

        pt_tiles = {}

        qk_ps = {}

        def qk_half(g, which, cs):
            ti = 0 if which == "q" else 1
            dst = qT if which == "q" else kT
            w_t = wqk_t[g][ti]
            bcol = g if which == "q" else NG + g
            if cs[0] == 0:
                qk_ps[(g, ti)] = [
                    pp.tile([P, 512], F32, tag="pp", name=f"qk{g}{ti}{i}")
                    for i in range(2)
                ]
            ps = qk_ps[(g, ti)]
            for c in cs:
                for qh in range(2):  # same lhsT back-to-back
                    nc.tensor.matmul(
                        out=ps[qh],
                        lhsT=w_t[:, c],
                        rhs=xT[:, c, qh * 512 : (qh + 1) * 512],
                        start=(c == 0),
                        stop=(c == ND - 1),
                    )
            if cs[-1] == ND - 1:
                for qh in range(2):
                    nc.vector.tensor_scalar_add(
                        out=dst[:, g, qh * 512 : (qh + 1) * 512],
                        in0=ps[qh],
                        scalar1=bqk_sb[:, bcol : bcol + 1],
                    )

        def v_proj(st):
            ps = [
                pp.tile([P, 512], F32, tag="pp", name=f"v{st}{i}")
                for i in range(2)
            ]
            for c in range(ND):
                for hh in range(2):  # same lhsT back-to-back
                    nc.tensor.matmul(
                        out=ps[hh],
                        lhsT=xT[:, c, st * P : (st + 1) * P],
                        rhs=wv_sb[:, c, hh * 512 : (hh + 1) * 512],
                        start=(c == 0),
                        stop=(c == ND - 1),
                    )
            for hh in range(2):
                nc.vector.tensor_add(
                    out=vb[:, st, hh * 8 : (hh + 1) * 8, 0:E],
                    in0=ps[hh].rearrange("p (h e) -> p h e", h=8),
                    in1=bvb_sb[:, hh * 512 : (hh + 1) * 512].rearrange(
                        "p (h e) -> p h e", h=8
                    ),
                )

        def sc(g, kt):
            # the two heads' matmuls sit in disjoint PE row groups -> the
            # qh pairs run concurrently; one N=2048 activation exps the
            # whole (pair, key-tile)
            stt = stp.tile([P, 2, 2, 512], F32, tag="st", name=f"st{g}{kt}")
            ptt = ptp.tile([P, 2, 2, 512], BF16, tag="pt", name=f"pt{g}{kt}")
            for qh in range(2):
                for h2 in range(2):
                    nc.tensor.matmul(
                        out=stt[:, qh, h2],
                        lhsT=kT[h2 * E : (h2 + 1) * E, g, kt * P : (kt + 1) * P],
                        rhs=qT[h2 * E : (h2 + 1) * E, g, qh * 512 : (qh + 1) * 512],
                        start=True,
                        stop=True,
                    )
            nc.scalar.activation(
                out=ptt,
                in_=stt,
                func=AF.Exp,
                bias=mb_sb[:, kt : kt + 1],
                scale=0.125,
            )
            pt_tiles[(g, kt)] = ptt

        pv_zp = {}

        def pv_mms(g, h2, kts):
            h = 2 * g + h2
            if kts[0] == 0:
                pv_zp[(g, h2)] = [
                    zpsp.tile([E + 1, 512], F32, tag="zp", name=f"zp{g}{h2}{i}")
                    for i in range(2)
                ]
            zp2 = pv_zp[(g, h2)]
            for kt in kts:
                for qh in range(2):  # same lhsT back-to-back
                    nc.tensor.matmul(
                        out=zp2[qh],
                        lhsT=vb[:, kt, h, :],
                        rhs=pt_tiles[(g, kt)][:, qh, h2],
                        start=(kt == 0),
                        stop=(kt == NS - 1),
                    )
            if kts[-1] == NS - 1:
                # evacuate PSUM promptly (cheap DVE copies with no DMA
                # dependency) so the next PV group's matmuls never wait on
                # the normalize chain
                zu = zup.tile([E + 1, 2, 512], BF16, tag="zu", name=f"zu{g}{h2}")
                with nc.allow_low_precision(reason="bf16 z and softmax denom"):
                    for qh in range(2):
                        nc.vector.tensor_copy(out=zu[:, qh], in_=zp2[qh])
                # normalize off the PE critical path: DVE reciprocal, then
                # broadcast+multiply on the otherwise-idle GpSimd engine
                rc = rcp.tile([1, 2, 512], BF16, tag="rc", name=f"rc{g}{h2}")
                with nc.allow_low_precision(reason="bf16 softmax denom"):
                    nc.vector.reciprocal(out=rc, in_=zu[E : E + 1])
                bc = bcp.tile([E, 2, 512], BF16, tag="bc", name=f"bc{g}{h2}")
                nc.sync.dma_start(
                    out=bc,
                    in_=rc.rearrange("p q x -> p (q x)")
                    .unsqueeze(1)
                    .broadcast_to((1, E, 1024))
                    .rearrange("p e (q x) -> p e q x", q=2),
                )
                nc.gpsimd.tensor_mul(
                    zT[h2 * E : (h2 + 1) * E, g, :].rearrange(
                        "p (q x) -> p q x", q=2
                    ),
                    zu[0:E],
                    bc,
                )

        def out_proj(st):
            ops = [
                pp.tile([P, 512], F32, tag="pp", name=f"op{st}{i}")
                for i in range(2)
            ]
            for g in range(NG):
                for dh in range(2):  # same lhsT back-to-back
                    nc.tensor.matmul(
                        out=ops[dh],
                        lhsT=zT[:, g, st * P : (st + 1) * P],
                        rhs=wo_sb[:, g, dh * 512 : (dh + 1) * 512],
                        start=(g == 0),
                        stop=(g == NG - 1),
                    )
            ob = obp.tile([P, D], F32, tag="ob", name=f"ob{st}")
            for dh in range(2):
                nc.vector.tensor_add(
                    out=ob[:, dh * 512 : (dh + 1) * 512],
                    in0=ops[dh],
                    in1=bob_sb[:, dh * 512 : (dh + 1) * 512],
                )
            nc.sync.dma_start(out=out_d[st * P : (st + 1) * P, :], in_=ob)

        # ---- pipelined schedule ----
        # pt pool-ring safety with bufs=12: a score unit sc(g, kt) reuses
        # the pt slot of sc(g-1, kt+4); every PV read of that slot is
        # emitted earlier in the iteration (PV chunks precede the score
        # units that recycle their tiles)
        C_LO = list(range(0, ND // 2))
        C_HI = list(range(ND // 2, ND))
        KT_LO = list(range(0, NS // 2))
        KT_HI = list(range(NS // 2, NS))
        qk_half(0, "q", C_LO)
        qk_half(0, "q", C_HI)
        qk_half(0, "k", C_LO)
        qk_half(0, "k", C_HI)
        load_pair(2)
        for kt in range(NS):  # iter 0: scores pair 0 + whole V projection
            sc(0, kt)
            v_proj(kt)
        qk_half(1, "q", C_LO)
        qk_half(1, "q", C_HI)
        qk_half(1, "k", C_LO)
        qk_half(1, "k", C_HI)
        for g in range(1, NG):
            if g + 2 <= NG - 1:
                load_pair(g + 2)
            last = g == NG - 1
            pv_mms(g - 1, 0, KT_LO)
            sc(g, 0)
            pv_mms(g - 1, 0, KT_HI)
            sc(g, 1)
            if not last:
                qk_half(g + 1, "q", C_LO)
            sc(g, 2)
            pv_mms(g - 1, 1, KT_LO)
            sc(g, 3)
            pv_mms(g - 1, 1, KT_HI)
            sc(g, 4)
            if not last:
                qk_half(g + 1, "q", C_HI)
            sc(g, 5)
            if not last:
                qk_half(g + 1, "k", C_LO)
            sc(g, 6)
            if not last:
                qk_half(g + 1, "k", C_HI)
            sc(g, 7)
        # tail: PV of the last pair chases its exps
        pv_mms(NG - 1, 0, KT_LO)
        pv_mms(NG - 1, 0, KT_HI)
        pv_mms(NG - 1, 1, KT_LO)
        pv_mms(NG - 1, 1, KT_HI)
        # out_proj's accumulation ends on pair 7, so each tile's first 14
        # matmuls can run while pair 7's normalize is still in flight
        for st in range(NS):
            out_proj(st)

    if split_waits:
        _split_multi_waits(nc)
    return nc


def _split_multi_waits(nc):
    """This walrus build encodes at most ONE sync wait per instruction.
    Tile emits more. Hoist excess waits onto same-engine EventSemaphore
    instructions inserted immediately before the offender - engines and
    DGE sequencers execute their streams in order, so this preserves
    semantics exactly."""
    n = 0
    for fn in nc.m.functions:
        for bb in fn.blocks:
            out = []
            for inst in bb.instructions:
                si = getattr(inst, "sync_info", None)
                waits = list(si.on_wait) if si is not None and si.on_wait else []
                if len(waits) > 1:
                    for w in waits[:-1]:
                        n += 1
                        out.append(
                            mybir.InstEventSemaphore(
                                name=f"evw-{n}",
                                engine=inst.engine,
                                sync_info=mybir.SyncInfo(
                                    on_wait=[w], on_update=[]
                                ),
                            )
                        )
                    si.on_wait = [waits[-1]]
                out.append(inst)
            bb.instructions[:] = out


_NC_CACHE = None


def _get_nc():
    global _NC_CACHE
    if _NC_CACHE is None:
        _NC_CACHE = build_program()
    return _NC_CACHE


def _make_in_maps(inputs):
    import ml_dtypes

    bf16 = ml_dtypes.bfloat16
    x = np.asarray(inputs["x"], np.float32)
    mask = np.asarray(inputs["key_attention_mask"])
    wq = np.asarray(inputs["W_Q"], np.float32).astype(bf16)
    wk = np.asarray(inputs["W_K"], np.float32).astype(bf16)
    wv = np.asarray(inputs["W_V"], np.float32).astype(bf16)
    wo = np.asarray(inputs["W_O"], np.float32).astype(bf16)
    bq = np.asarray(inputs["b_Q"], np.float32)  # (H, E)
    bk = np.asarray(inputs["b_K"], np.float32)
    bv = np.asarray(inputs["b_V"], np.float32)
    bo = np.asarray(inputs["b_O"], np.float32)  # (D,)

    def pack_qk(w):  # (H, D, E) -> [p, g, c, (h2 e)]
        return np.ascontiguousarray(
            w.reshape(NG, 2, ND, P, E).transpose(3, 0, 2, 1, 4).reshape(P, NG, ND, P)
        )

    # per-partition (half*64+e) bias columns per pair
    def pack_b(b):  # (H, E) -> [p, g]
        return np.ascontiguousarray(
            b.reshape(NG, 2, E).transpose(1, 2, 0).reshape(P, NG)
        )

    bqk = np.concatenate([pack_b(bq), pack_b(bk)], axis=1).astype(np.float32)
    shared = {
        "wq": pack_qk(wq),
        "wk": pack_qk(wk),
        # (H, D, E) -> [p, c, (h e)]
        "wv": np.ascontiguousarray(
            wv.reshape(H, ND, P, E).transpose(2, 1, 0, 3).reshape(P, ND, H * E)
        ),
        # (H, E, D) -> [(h2 e), g, d]
        "wo": np.ascontiguousarray(
            wo.reshape(NG, 2, E, D).transpose(1, 2, 0, 3).reshape(P, NG, D)
        ),
        "bqk": bqk,
        "bvb": np.ascontiguousarray(
            np.tile(bv.reshape(1, H * E), (P, 1))
        ).astype(bf16),
        "bob": np.ascontiguousarray(np.tile(bo.reshape(1, D), (P, 1))).astype(
            bf16
        ),
    }
    in_maps = []
    for b in range(B):
        m = dict(shared)
        xt = x[b].T.astype(bf16)  # (D, S) -> [p, c, s]
        m["xt"] = np.ascontiguousarray(
            xt.reshape(ND, P, S).transpose(1, 0, 2)
        )
        mb = ((mask[b] != 0).astype(np.float32) - 1.0) * MASK_NEG
        m["mb"] = np.ascontiguousarray(mb.reshape(NS, P).T)
        in_maps.append(m)
    return in_maps


def run(inputs, trace=False):
    nc = _get_nc()
    res = run_bass_kernel_spmd(nc, _make_in_maps(inputs), list(range(B)),
                               trace=trace)
    out = np.stack([res.results[b]["out"] for b in range(B)], axis=0)
    return out, res


def kernel(**inputs) -> np.ndarray:
    out, _ = run(inputs, trace=False)
    return out
